# revision 1
# baseline (speedup 1.0000x reference)
"""Distance-correlation (DcorLoss) kernel for 8 trn2 NeuronCores.

Math: for x, y [n=8192, d=128]:
  a = pairwise_dist(x), b = pairwise_dist(y)   (n x n, symmetric, zero diag)
  A = double_center(a), B = double_center(b)
  dcor = -sqrt(sum(A*B)) / sqrt(sqrt(sum(A*A)) * sqrt(sum(B*B)))   (n^2 factors cancel)

Key identities (never materialize A/B):
  sum(HaH o HbH) = sum(at o bt) - 2/n * dot(rs_at, rs_bt) + sum(at)*sum(bt)/n^2
for at = a - mu (any constant shift; double centering annihilates it). The
mu ~ E[dist] shift keeps fp32 device accumulations well-conditioned. And the
squared-distance Frobenius norms have a closed form -- no elementwise pass:
  sum_ij dist^2_ij = 2n * sum_i |x_i|^2 - 2 |sum_i x_i|^2
so only sum (a-mu)*b needs streaming; sum(a-mu)^2 / sum(b-mu)^2 come from
row sums + norms + the column-sum vector of x. Cross-core combining is fp64
on host (the only inter-core step; partials are tiny).

Sharding: block-rows; core c owns rows [c*1024, (c+1)*1024), streams all columns.

Per (128-row x 1024-col) tile pair, the device computes:
  PE:   psum = -2*x_blk^T x (K=128, bf16) + ones2 (x) [n_hi; n_lo] (K=2 bf16
        hi/lo split of the fp32 column norms -> ~16-bit mantissa)
  DVE:  psum[diag block] += mu^2 * I  (data-driven: per-core `diagsel` input
        is nonzero only on the core's own diagonal window)
  ACT:  t = sqrt(psum + n_i)  [per-partition fp32 bias], accum_out -> row sums
  DVE:  (t_a - mu) * t_b -> accum_out   (one scalar_tensor_tensor)
Forcing the diagonal of sq to mu^2 keeps sqrt NaN-free; host replaces the known
diagonal contribution exactly (true diag of a is 0).
"""

import os

import numpy as np

import concourse.bass as bass
import concourse.tile as tile
from concourse import bacc, mybir
from concourse.bass_utils import run_bass_kernel_spmd

P = 128            # partitions / d
N = 8192           # points
NCORES = 8
BLK = N // NCORES  # 1024 rows per core
CI_N = BLK // P    # 8 row chunks per core
W = 1024           # column window
JT_N = N // W      # 8 column windows
MU = 16.0          # ~E[pairwise dist] for randn d=128; any constant is exact math
MU2 = MU * MU
RES_W = 48

_programs = {}


def _build(mm_mode: str):
    """mm_mode: 'bf16' | 'f32' (matmul operand dtype)."""
    dt = mybir.dt
    f32 = dt.float32
    mmdt = dt.bfloat16 if mm_mode == "bf16" else dt.float32
    A = mybir.AluOpType
    AF = mybir.ActivationFunctionType

    nc = bacc.Bacc("TRN2", target_bir_lowering=False, debug=False,
                   num_devices=NCORES)

    dxT = nc.dram_tensor("xT", [P, N], f32, kind="ExternalInput").ap()
    dyT = nc.dram_tensor("yT", [P, N], f32, kind="ExternalInput").ap()
    dxb = nc.dram_tensor("xblkT", [P, BLK], f32, kind="ExternalInput").ap()
    dyb = nc.dram_tensor("yblkT", [P, BLK], f32, kind="ExternalInput").ap()
    ddg = nc.dram_tensor("diagsel", [P, JT_N * P], f32, kind="ExternalInput").ap()
    dew = nc.dram_tensor("eyewide", [P, 4 * 512], f32, kind="ExternalInput").ap()
    dout = nc.dram_tensor("out", [P, RES_W], f32, kind="ExternalOutput").ap()

    with tile.TileContext(nc) as tc:
        with tc.tile_pool(name="const", bufs=1) as cp, \
             tc.tile_pool(name="psum", bufs=1, space="PSUM") as pp, \
             tc.tile_pool(name="ab", bufs=3) as abp, \
             tc.tile_pool(name="trd", bufs=2) as trd:

            # ── persistent operands ────────────────────────────────────
            xTc = cp.tile([P, N], mmdt, tag="xTc")
            yTc = cp.tile([P, N], mmdt, tag="yTc")
            xblk2 = cp.tile([P, BLK], mmdt, tag="xblk2")
            yblk2 = cp.tile([P, BLK], mmdt, tag="yblk2")
            # column norms as bf16 hi/lo rows: n_j = hi_j + lo_j (K=2 matmul)
            nfhl_x = cp.tile([2, N], mmdt, tag="nfhl_x")
            nfhl_y = cp.tile([2, N], mmdt, tag="nfhl_y")
            nbx = cp.tile([P, CI_N], f32, tag="nbx")
            nby = cp.tile([P, CI_N], f32, tag="nby")
            diag_m = cp.tile([P, JT_N * P], mmdt, tag="diag_m")
            eyew_m = cp.tile([P, 4 * 512], mmdt, tag="eyew_m")
            ones2 = cp.tile([2, P], mmdt, tag="ones2")
            nc.vector.memset(ones2[:], 1.0)
            onesc = cp.tile([P, 1], mmdt, tag="onesc")
            nc.vector.memset(onesc[:], 1.0)
            quarc = cp.tile([P, 1], f32, tag="quarc")
            nc.vector.memset(quarc[:], 0.25)

            res = cp.tile([P, RES_W], f32, tag="res")
            nc.vector.memset(res[:], 0.0)

            # PE warm-up: ~5us of dense back-to-back matmuls on constant data
            # so the HAM clock-gate reaches 8/8 before real work starts (cold
            # first executions otherwise run the whole kernel at half clock)
            wur = cp.tile([2, 512], mmdt, tag="wur")
            nc.vector.memset(wur[:], 0.0)
            wt = pp.tile([P, W], f32, tag="a", bufs=2)
            for _ in range(24):
                nc.tensor.matmul(wt[:, 0:512], ones2[:], wur[:],
                                 start=True, stop=True)

            # ── setup (scratch pool closes before the main loop) ──────
            with tc.tile_pool(name="setup", bufs=1) as sp:
                big = sp.tile([P, N], f32, tag="big")
                nc.sync.dma_start(big[:], dxT[:])
                nc.scalar.copy(xTc[:], big[:])
                big2 = sp.tile([P, N], f32, tag="big")
                nc.sync.dma_start(big2[:], dyT[:])
                nc.scalar.copy(yTc[:], big2[:])

                for dsrc, dst in ((dxb, xblk2), (dyb, yblk2)):
                    raw = sp.tile([P, BLK], f32, tag="braw")
                    nc.sync.dma_start(raw[:], dsrc[:])
                    nc.scalar.mul(dst[:], raw[:], -2.0)

                dgr = sp.tile([P, JT_N * P], f32, tag="dgr")
                nc.sync.dma_start(dgr[:], ddg[:])
                nc.scalar.copy(diag_m[:], dgr[:])
                ewr = sp.tile([P, 4 * 512], f32, tag="ewr")
                nc.sync.dma_start(ewr[:], dew[:])
                nc.scalar.copy(eyew_m[:], ewr[:])

                # full column norms nf[j] = sum_d xTc[d, j]^2 (fp32 in PSUM),
                # split per-slice into bf16 hi/lo; DMA does the partition
                # placement into nfhl (engines are lane-aligned)
                for src, nfhl in ((xTc, nfhl_x), (yTc, nfhl_y)):
                    hi = sp.tile([1, N], mmdt, tag="hi")
                    lo = sp.tile([1, N], mmdt, tag="lo")
                    for j8 in range(JT_N):
                        sq = sp.tile([P, W], mmdt, tag="sq", bufs=2)
                        nc.scalar.activation(sq[:], src[:, bass.ts(j8, W)],
                                             AF.Square)
                        for h in range(2):
                            ps = pp.tile([P, W], f32, tag="a", bufs=2)
                            sl = bass.ds(j8 * W + h * 512, 512)
                            nc.tensor.matmul(ps[0:1, 0:512], onesc[:],
                                             sq[:, bass.ts(h, 512)],
                                             start=True, stop=True)
                            nc.vector.tensor_copy(hi[0:1, sl],
                                                  ps[0:1, 0:512])
                            nc.vector.tensor_tensor(lo[0:1, sl],
                                                    ps[0:1, 0:512],
                                                    hi[0:1, sl],
                                                    op=A.subtract)
                        # ship this window now so main-loop iterations can
                        # start before the whole norms row is built
                        wsl = bass.ts(j8, W)
                        nc.sync.dma_start(nfhl[0:1, wsl], hi[0:1, wsl])
                        nc.sync.dma_start(nfhl[1:2, wsl], lo[0:1, wsl])

                # block norms as [P, CI_N] fp32 columns (sqrt bias):
                # (-2x)^2 * 0.25 = x^2
                for src, dst in ((xblk2, nbx), (yblk2, nby)):
                    sqb = sp.tile([P, BLK], f32, tag="sqb")
                    nc.scalar.activation(sqb[:], src[:], AF.Square)
                    for ci in range(CI_N):
                        ps = pp.tile([P, W], f32, tag="a", bufs=2)
                        nc.tensor.matmul(ps[:, 0:1], sqb[:, bass.ts(ci, P)],
                                         quarc[:], start=True, stop=True)
                        nc.vector.tensor_copy(dst[:, ci:ci + 1], ps[:, 0:1])

            # closed-form helpers: sum of norms (hi+lo rows reduced) and the
            # column-sum vector s = sum_i x_i, both over the bf16 values
            nc.vector.tensor_reduce(res[0:2, 41:42], nfhl_x[:, :],
                                    axis=mybir.AxisListType.X, op=A.add)
            nc.vector.tensor_reduce(res[0:2, 42:43], nfhl_y[:, :],
                                    axis=mybir.AxisListType.X, op=A.add)
            nc.vector.tensor_reduce(res[:, 44:45], xTc[:, :],
                                    axis=mybir.AxisListType.X, op=A.add)
            nc.vector.tensor_reduce(res[:, 45:46], yTc[:, :],
                                    axis=mybir.AxisListType.X, op=A.add)

            # ── stages ────────────────────────────────────────────────
            st = [cp.tile([P, CI_N * JT_N], f32, tag=f"st{q}", name=f"st{q}")
                  for q in range(3)]

            # ── main loop ─────────────────────────────────────────────
            for ci in range(CI_N):
                for jt in range(JT_N):
                    col = ci * JT_N + jt
                    h_diag = ci // 4
                    psA = pp.tile([P, W], f32, tag="a", bufs=2)
                    psB = pp.tile([P, W], f32, tag="b", bufs=2)
                    # weight-grouped order: mains (xblk2 / yblk2), then all
                    # norm matmuls (shared ones2 weights), then the diag eye
                    for ps_, blk2, full in ((psA, xblk2, xTc),
                                            (psB, yblk2, yTc)):
                        for h in range(2):
                            nc.tensor.matmul(
                                ps_[:, bass.ds(h * 512, 512)],
                                blk2[:, bass.ts(ci, P)],
                                full[:, bass.ds(jt * W + h * 512, 512)],
                                start=True, stop=False)
                    for ps_, nfhl in ((psA, nfhl_x), (psB, nfhl_y)):
                        for h in range(2):
                            nc.tensor.matmul(
                                ps_[:, bass.ds(h * 512, 512)], ones2[:],
                                nfhl[:, bass.ds(jt * W + h * 512, 512)],
                                start=False, stop=(h != h_diag))
                    for ps_ in (psA, psB):
                        # += mu^2*I on the diag window (zeros unless jt ==
                        # core id): (mu I)^T (mu I @ offset)
                        nc.tensor.matmul(ps_[:, bass.ds(h_diag * 512, 512)],
                                         diag_m[:, bass.ts(jt, P)],
                                         eyew_m[:, bass.ts(ci % 4, 512)],
                                         start=False, stop=True)

                    aT = abp.tile([P, W], f32, tag="a")
                    bT = abp.tile([P, W], f32, tag="b")
                    nc.scalar.activation(aT[:], psA[:], AF.Sqrt,
                                         bias=nbx[:, ci:ci + 1],
                                         accum_out=st[0][:, col:col + 1])
                    nc.scalar.activation(bT[:], psB[:], AF.Sqrt,
                                         bias=nby[:, ci:ci + 1],
                                         accum_out=st[1][:, col:col + 1])
                    t0 = trd.tile([P, W], f32, tag="t")
                    nc.vector.scalar_tensor_tensor(
                        t0[:], aT[:], MU, bT[:], op0=A.subtract, op1=A.mult,
                        accum_out=st[2][:, col:col + 1])

            # ── epilogue ──────────────────────────────────────────────
            nc.vector.tensor_copy(res[:, 24:24 + CI_N], nbx[:, :])
            nc.vector.tensor_copy(res[:, 32:32 + CI_N], nby[:, :])
            for q in range(3):
                for ci in range(CI_N):
                    o = q * CI_N + ci
                    nc.vector.tensor_reduce(res[:, o:o + 1],
                                            st[q][:, bass.ts(ci, JT_N)],
                                            axis=mybir.AxisListType.X,
                                            op=A.add)
            nc.sync.dma_start(dout[:], res[:])

    nc.compile()
    return nc


def _get_program(mm_mode: str):
    if mm_mode not in _programs:
        _programs[mm_mode] = _build(mm_mode)
    return _programs[mm_mode]


def make_in_maps(x: np.ndarray, y: np.ndarray):
    x = np.ascontiguousarray(np.asarray(x, np.float32))
    y = np.ascontiguousarray(np.asarray(y, np.float32))
    xT = np.ascontiguousarray(x.T)
    yT = np.ascontiguousarray(y.T)
    eye = (np.eye(P, dtype=np.float32) * MU)
    ew = np.zeros((P, 4 * 512), np.float32)
    for k in range(4):
        for p in range(P):
            ew[p, k * 512 + k * P + p] = MU
    in_maps = []
    for c in range(NCORES):
        dg = np.zeros((P, JT_N * P), np.float32)
        dg[:, c * P:(c + 1) * P] = eye
        in_maps.append({
            "xT": xT,
            "yT": yT,
            "xblkT": np.ascontiguousarray(x[c * BLK:(c + 1) * BLK].T),
            "yblkT": np.ascontiguousarray(y[c * BLK:(c + 1) * BLK].T),
            "diagsel": dg,
            "eyewide": ew,
        })
    return in_maps


def finalize(outs):
    """outs: list of 8 [128, 48] arrays -> scalar dcor (fp64 host math).

    Cols: rsa 0:8 | rsb 8:16 | pab 16:24 | [0:2,41]=(sum hi, sum lo) of x
    norms | [0:2,42]= same for y | [:,44]=sum_i x_i | [:,45]=sum_i y_i.
    Device row sums include the forced diag ~mu (true diag of a is 0).
    """
    n = float(N)
    rs_a = np.empty(N, np.float64)
    rs_b = np.empty(N, np.float64)
    pab = 0.0
    for c, o in enumerate(outs):
        o = np.asarray(o, np.float64)
        rs_a[c * BLK:(c + 1) * BLK] = o[:, 0:CI_N].T.ravel()
        rs_b[c * BLK:(c + 1) * BLK] = o[:, CI_N:2 * CI_N].T.ravel()
        pab += o[:, 2 * CI_N:3 * CI_N].sum()

    o0 = np.asarray(outs[0], np.float64)
    # column-norm sums as the device's K=2 matmul sees them (bf16 hi+lo of
    # bf16-rounded squares); row-bias norms are the fp32-exact path
    sum_nxc = o0[0, 41] + o0[1, 41]
    sum_nyc = o0[0, 42] + o0[1, 42]
    sum_nxr = sum(np.asarray(o, np.float64)[:, 24:24 + CI_N].sum()
                  for o in outs)
    sum_nyr = sum(np.asarray(o, np.float64)[:, 32:32 + CI_N].sum()
                  for o in outs)
    sx = o0[:, 44]                        # sum_i x_i  [128]
    sy = o0[:, 45]
    # closed-form squared-distance Frobenius norms, consistent with the
    # device's mixed n_i/n_j paths (true zero diag):
    sq_a = n * (sum_nxr + sum_nxc) - 2.0 * np.dot(sx, sx)   # sum_ij a_ij^2
    sq_b = n * (sum_nyr + sum_nyc) - 2.0 * np.dot(sy, sy)

    sa = rs_a - MU          # true (zero-diag) row sums of a
    sb = rs_b - MU
    sat = sa - n * MU       # row sums of (a - mu)
    sbt = sb - n * MU
    Ua = sat.sum()
    Ub = sbt.sum()
    # device pab = sum (a-mu)*b (diag contributes ~0 in device and truth)
    Sab = pab - MU * (sa.sum() - MU * n * n)
    Saa = sq_a - 2.0 * MU * sa.sum() + MU2 * n * n
    Sbb = sq_b - 2.0 * MU * sb.sum() + MU2 * n * n

    sumAB = Sab - 2.0 * np.dot(sat, sbt) / n + Ua * Ub / n**2
    sumAA = Saa - 2.0 * np.dot(sat, sat) / n + Ua * Ua / n**2
    sumBB = Sbb - 2.0 * np.dot(sbt, sbt) / n + Ub * Ub / n**2

    inv_n2 = 1.0 / (n * n)
    dcov2_xy = sumAB * inv_n2
    dcov2_xx = sumAA * inv_n2
    dcov2_yy = sumBB * inv_n2
    dcor = -np.sqrt(dcov2_xy) / np.sqrt(np.sqrt(dcov2_xx) * np.sqrt(dcov2_yy))
    return np.asarray(dcor, dtype=np.float32)


def run(x, y, mm_mode=None, trace=False, tmpdir=None):
    if mm_mode is None:
        mm_mode = os.environ.get("DCOR_MM", "bf16")
    nc = _get_program(mm_mode)
    in_maps = make_in_maps(x, y)
    res = run_bass_kernel_spmd(nc, in_maps, core_ids=list(range(NCORES)),
                               trace=trace, tmpdir=tmpdir)
    outs = [r["out"] for r in res.results]
    return finalize(outs), res


def kernel(x, y):
    val, _ = run(x, y)
    return val



# revision 3
# speedup vs baseline: 1.4899x; 1.4899x over previous
"""Distance-correlation (DcorLoss) kernel for 8 trn2 NeuronCores.

Math: for x, y [n=8192, d=128]:
  a = pairwise_dist(x), b = pairwise_dist(y)   (n x n, symmetric, zero diag)
  A = double_center(a), B = double_center(b)
  dcor = -sqrt(sum(A*B)) / sqrt(sqrt(sum(A*A)) * sqrt(sum(B*B)))

Never materialize A/B:
  sum(A o B) = sum(a o b) - 2/n dot(rs_a, rs_b) + sum(a) sum(b) / n^2
and the squared-distance Frobenius norms have a closed form (host, exact):
  sum_ij dist^2 = 2n sum_i |x_i|^2 - 2 |sum_i x_i|^2
so the device only streams: row sums of a and b (ACT accum) and
sum (a - mu) * b (DVE accum). Everything else is host fp64.

Device work per (128-row x 1024-col) tile pair:
  PE:  psum = (-2 x_blk)^T x  (K=128 bf16)  + ones2 x [n_hi; n_lo] (K=2,
       bf16 hi/lo split of fp32 column norms)
  ACT: t = sqrt(psum + n_i)  [per-partition fp32 bias], accum -> row sums
  DVE: (t_a - mu) * t_b -> accum (sum of products)

All operand prep is host-side: inputs arrive as bf16 (or f32 for biases)
in final layout; no on-device casts / norm computation / reductions.

Sharding: block-rows, with per-core COLUMN ROTATION: core c's column j is
global column (j + c*1024) mod n. Rotation puts each core's diagonal block
at local window 0, so the mu^2-diagonal fix (keeps sqrt NaN-free on the
~0 diagonal) costs 2 matmuls only on the 8 window-0 tiles, and the SPMD
program is identical across cores.
"""

import numpy as np
import ml_dtypes

import concourse.bass as bass
import concourse.tile as tile
from concourse import bacc, mybir
from concourse.bass_utils import run_bass_kernel_spmd

P = 128            # partitions / d
N = 8192           # points
NCORES = 8
BLK = N // NCORES  # 1024 rows per core
CI_N = BLK // P    # 8 row chunks per core
W = 1024           # column window
JT_N = N // W      # 8 column windows
MU = 16.0          # ~E[pairwise dist] for randn d=128; any constant is exact
RES_W = 24

BF16 = ml_dtypes.bfloat16

_programs = {}


NW_SYM = 5         # sym mode: windows 0..4 (diag + 4 cyclic) per core
NCOL = N  # dram moving-tensor width (full; sym mode reads first 5 windows)


def _build(mode: str):
    dt = mybir.dt
    f32 = dt.float32
    bf = dt.bfloat16
    A = mybir.AluOpType
    AF = mybir.ActivationFunctionType

    nc = bacc.Bacc("TRN2", target_bir_lowering=False, debug=False,
                   num_devices=NCORES)

    dxT = nc.dram_tensor("xT", [P, N], bf, kind="ExternalInput").ap()
    dyT = nc.dram_tensor("yT", [P, N], bf, kind="ExternalInput").ap()
    dxb = nc.dram_tensor("xblk2", [P, BLK], bf, kind="ExternalInput").ap()
    dyb = nc.dram_tensor("yblk2", [P, BLK], bf, kind="ExternalInput").ap()
    dnfx = nc.dram_tensor("nfx", [2, N], bf, kind="ExternalInput").ap()
    dnfy = nc.dram_tensor("nfy", [2, N], bf, kind="ExternalInput").ap()
    dnbx = nc.dram_tensor("nbx", [P, CI_N], f32, kind="ExternalInput").ap()
    dnby = nc.dram_tensor("nby", [P, CI_N], f32, kind="ExternalInput").ap()
    deye = nc.dram_tensor("eye128", [P, P], bf, kind="ExternalInput").ap()
    dew = nc.dram_tensor("eyewide", [P, 4 * 512], bf, kind="ExternalInput").ap()
    dout = nc.dram_tensor("out", [P, RES_W], f32, kind="ExternalOutput").ap()

    with tile.TileContext(nc) as tc:
        with tc.tile_pool(name="const", bufs=1) as cp, \
             tc.tile_pool(name="psum", bufs=3, space="PSUM") as pp, \
             tc.tile_pool(name="ab", bufs=3) as abp, \
             tc.tile_pool(name="trd", bufs=2) as trd:

            # ── persistent operands, DMA'd in final dtype/layout ──────
            xTc = cp.tile([P, N], bf, tag="xTc")
            yTc = cp.tile([P, N], bf, tag="yTc")
            xblk2 = cp.tile([P, BLK], bf, tag="xblk2")
            yblk2 = cp.tile([P, BLK], bf, tag="yblk2")
            nfx = cp.tile([2, N], bf, tag="nfx")
            nfy = cp.tile([2, N], bf, tag="nfy")
            nbx = cp.tile([P, CI_N], f32, tag="nbx")
            nby = cp.tile([P, CI_N], f32, tag="nby")
            eye128 = cp.tile([P, P], bf, tag="eye128")
            eyew = cp.tile([P, 4 * 512], bf, tag="eyew")
            ones2 = cp.tile([2, P], bf, tag="ones2")
            nc.vector.memset(ones2[:], 1.0)

            res = cp.tile([P, RES_W], f32, tag="res")
            nc.vector.memset(res[:], 0.0)

            st = [cp.tile([P, CI_N * JT_N], f32, tag=f"st{q}", name=f"st{q}")
                  for q in range(3)]

            # PE warm-up on constant data: release the HAM clock-gate
            # before real matmuls start (cold runs stream at 1.2 GHz).
            wur = cp.tile([2, 512], bf, tag="wur")
            nc.vector.memset(wur[:], 0.0)
            for _ in range(24):
                wt = pp.tile([P, W], f32, tag="ps")
                nc.tensor.matmul(wt[:, 0:512], ones2[:], wur[:],
                                 start=True, stop=True)

            # small operands first, then per-window slices of the big
            # moving tensors so window-0 compute starts ASAP
            nc.sync.dma_start(xblk2[:], dxb[:])
            nc.sync.dma_start(yblk2[:], dyb[:])
            nc.sync.dma_start(nbx[:], dnbx[:])
            nc.sync.dma_start(nby[:], dnby[:])
            nc.sync.dma_start(eye128[:], deye[:])
            nc.sync.dma_start(eyew[:], dew[:])
            nc.sync.dma_start(nfx[:], dnfx[:])
            nc.sync.dma_start(nfy[:], dnfy[:])
            for w in range(JT_N):
                sl = bass.ts(w, W)
                nc.sync.dma_start(xTc[:, sl], dxT[:, sl])
                nc.sync.dma_start(yTc[:, sl], dyT[:, sl])

            # ── main loop ─────────────────────────────────────────────
            for w in range(JT_N):
                for ci in range(CI_N):
                    col = ci * JT_N + w
                    h_diag = ci // 4
                    psA = pp.tile([P, W], f32, tag="ps")
                    psB = pp.tile([P, W], f32, tag="ps")
                    # mains (grouped by weights), then norm rows (shared
                    # ones2 weights), then diag fix (window 0 only)
                    for ps_, blk2, full in ((psA, xblk2, xTc),
                                            (psB, yblk2, yTc)):
                        for h in range(2):
                            nc.tensor.matmul(
                                ps_[:, bass.ds(h * 512, 512)],
                                blk2[:, bass.ts(ci, P)],
                                full[:, bass.ds(w * W + h * 512, 512)],
                                start=True, stop=False)
                    for ps_, nf in ((psA, nfx), (psB, nfy)):
                        for h in range(2):
                            last = not (w == 0 and h == h_diag)
                            nc.tensor.matmul(
                                ps_[:, bass.ds(h * 512, 512)], ones2[:],
                                nf[:, bass.ds(w * W + h * 512, 512)],
                                start=False, stop=last)
                    if w == 0:
                        for ps_ in (psA, psB):
                            nc.tensor.matmul(
                                ps_[:, bass.ds(h_diag * 512, 512)],
                                eye128[:],
                                eyew[:, bass.ts(ci % 4, 512)],
                                start=False, stop=True)

                    aT = abp.tile([P, W], f32, tag="a")
                    bT = abp.tile([P, W], f32, tag="b")
                    nc.scalar.activation(aT[:], psA[:], AF.Sqrt,
                                         bias=nbx[:, ci:ci + 1],
                                         accum_out=st[0][:, col:col + 1])
                    nc.scalar.activation(bT[:], psB[:], AF.Sqrt,
                                         bias=nby[:, ci:ci + 1],
                                         accum_out=st[1][:, col:col + 1])
                    t0 = trd.tile([P, W], bf, tag="t")
                    nc.vector.scalar_tensor_tensor(
                        t0[:], aT[:], MU, bT[:], op0=A.subtract, op1=A.mult,
                        accum_out=st[2][:, col:col + 1])

            # ── epilogue: fold windows, ship result ───────────────────
            for q in range(3):
                for ci in range(CI_N):
                    o = q * CI_N + ci
                    nc.vector.tensor_reduce(res[:, o:o + 1],
                                            st[q][:, bass.ts(ci, JT_N)],
                                            axis=mybir.AxisListType.X,
                                            op=A.add)
            nc.sync.dma_start(dout[:], res[:])

    nc.compile()
    return nc


def _get_program(mode: str):
    if mode not in _programs:
        _programs[mode] = _build(mode)
    return _programs[mode]


def make_in_maps(x: np.ndarray, y: np.ndarray):
    x = np.asarray(x, np.float32)
    y = np.asarray(y, np.float32)
    xb = x.astype(BF16)
    yb = y.astype(BF16)

    # column norms of the bf16 points, fp32-exact, split bf16 hi/lo
    def norms_hi_lo(vb):
        n64 = (vb.astype(np.float64) ** 2).sum(axis=1)
        hi = n64.astype(BF16)
        lo = (n64 - hi.astype(np.float64)).astype(BF16)
        return n64, np.stack([hi, lo]).astype(BF16)

    nx64, nfx = norms_hi_lo(xb)
    ny64, nfy = norms_hi_lo(yb)

    xT = np.ascontiguousarray(xb.T)           # [128, 8192] bf16
    yT = np.ascontiguousarray(yb.T)
    xT2 = np.concatenate([xT, xT], axis=1)    # for cheap rotation slicing
    yT2 = np.concatenate([yT, yT], axis=1)
    nfx2 = np.concatenate([nfx, nfx], axis=1)
    nfy2 = np.concatenate([nfy, nfy], axis=1)

    eye = (np.eye(P) * MU).astype(BF16)
    ew = np.zeros((P, 4 * 512), BF16)
    for k in range(4):
        for p in range(P):
            ew[p, k * 512 + k * P + p] = BF16(MU)

    in_maps = []
    for c in range(NCORES):
        o = c * BLK
        xTr = np.ascontiguousarray(xT2[:, o:o + N])
        yTr = np.ascontiguousarray(yT2[:, o:o + N])
        in_maps.append({
            "xT": xTr,
            "yT": yTr,
            "xblk2": np.ascontiguousarray(BF16(-2.0) * xTr[:, 0:BLK]),
            "yblk2": np.ascontiguousarray(BF16(-2.0) * yTr[:, 0:BLK]),
            "nfx": np.ascontiguousarray(nfx2[:, o:o + N]),
            "nfy": np.ascontiguousarray(nfy2[:, o:o + N]),
            "nbx": np.ascontiguousarray(
                nx64[o:o + BLK].reshape(CI_N, P).T.astype(np.float32)),
            "nby": np.ascontiguousarray(
                ny64[o:o + BLK].reshape(CI_N, P).T.astype(np.float32)),
            "eye128": eye,
            "eyewide": ew,
        })
    host = {"xb64": xb.astype(np.float64), "yb64": yb.astype(np.float64)}
    return in_maps, host


def finalize(outs, host):
    """outs: 8 x [128, 24] f32 -> scalar dcor (host fp64).

    Cols per core: rs_a 0:8 | rs_b 8:16 | pab 16:24, laid out [p, ci] for
    global row c*1024 + ci*128 + p. Device row sums include the forced
    diagonal entry sqrt(mu^2) = mu (true diag of a distance matrix is 0).
    """
    n = float(N)
    rs_a = np.empty(N, np.float64)
    rs_b = np.empty(N, np.float64)
    pab = 0.0
    for c, o in enumerate(outs):
        o = np.asarray(o, np.float64)
        rs_a[c * BLK:(c + 1) * BLK] = o[:, 0:CI_N].T.ravel()
        rs_b[c * BLK:(c + 1) * BLK] = o[:, CI_N:2 * CI_N].T.ravel()
        pab += o[:, 2 * CI_N:3 * CI_N].sum()

    sa = rs_a - MU             # true (zero-diag) row sums of a
    sb = rs_b - MU
    Ra = sa.sum()
    Rb = sb.sum()

    # exact Frobenius norms of the bf16-point distance matrices
    def sq_frob(v64):
        s = v64.sum(axis=0)
        return 2.0 * n * (v64 * v64).sum() - 2.0 * np.dot(s, s)

    sq_a = sq_frob(host["xb64"])
    sq_b = sq_frob(host["yb64"])

    # device pab = sum (a - mu) * b; forced diag contributes (mu-mu)*mu = 0,
    # matching the true diag contribution (0 - mu) * 0 = 0 exactly.
    sum_ab = pab + MU * Rb

    sumAB = sum_ab - 2.0 * np.dot(sa, sb) / n + Ra * Rb / n**2
    sumAA = sq_a - 2.0 * np.dot(sa, sa) / n + Ra * Ra / n**2
    sumBB = sq_b - 2.0 * np.dot(sb, sb) / n + Rb * Rb / n**2

    inv_n2 = 1.0 / (n * n)
    dcor = (-np.sqrt(sumAB * inv_n2)
            / np.sqrt(np.sqrt(sumAA * inv_n2) * np.sqrt(sumBB * inv_n2)))
    return np.asarray(dcor, dtype=np.float32)


def run(x, y, mm_mode=None, trace=False, tmpdir=None):
    nc = _get_program("bf16")
    in_maps, host = make_in_maps(x, y)
    res = run_bass_kernel_spmd(nc, in_maps, core_ids=list(range(NCORES)),
                               trace=trace, tmpdir=tmpdir)
    outs = [r["out"] for r in res.results]
    return finalize(outs, host), res


def kernel(x, y):
    val, _ = run(x, y)
    return val
